# revision 29
# baseline (speedup 1.0000x reference)
"""Binarized 3x3 conv (BConv) on 8 TRN2 NeuronCores.

Reference computes: y = conv2d(x, sign(w), stride 1, pad 1) * scale[oc]
with x (32,256,56,56) f32, w (256*256*3*3,1) f32, scale (1,256,1,1) f32.

Strategy: data-parallel over batch (4 images per core, weights + scale
replicated). Per core the conv is lowered to matmuls on the PE array:
for each (ic_chunk, kh, kw), out[oc, pix] += w[ic, oc].T @ x[ic, pix_shifted],
18 accumulating matmuls per PSUM tile of 448 pixels (8 rows x 56).
Inputs are rounded to fp32r on-device (full PE rate at ~2^-13 precision;
binary +-1 weights are exact). Per-out-channel scale is applied by the
ScalarE Copy activation during PSUM evacuation.

Input DMAs issue on the sync engine, output DMAs on the scalar engine so
image prefetch never queues behind result stores. The PE is kept warm
through the initial load phase with throwaway small matmuls.
"""
import numpy as np

import concourse.bacc as bacc
import concourse.mybir as mybir
import concourse.tile as tile
from concourse.bass_utils import run_bass_kernel_spmd

N, IC, OC, H, W = 32, 256, 256, 56, 56
NCORES = 8
IMGS = N // NCORES          # 4 images per core
NCH = IC // 128             # 2 in-channel chunks
OCH = OC // 128             # 2 out-channel chunks
HP, WP = H + 2, W + 2       # padded 58x58
RT = 8                      # output rows per tile
PT = H // RT                # 7 row tiles
NPIX = RT * W               # 448 pixels per matmul
NWARM = 70                  # PE warmup matmuls bridging the load phase

_CACHE = {}


def _build():
    if "nc" in _CACHE:
        return _CACHE["nc"]
    f32 = mybir.dt.float32
    f32r = mybir.dt.float32r
    nc = bacc.Bacc("TRN2", target_bir_lowering=False, debug=False,
                   num_devices=NCORES)

    x_d = nc.declare_dram_parameter("x", [IMGS, NCH, 128, HP, WP], f32,
                                    isOutput=False)
    w_d = nc.declare_dram_parameter("w", [NCH, 3, 3, 128, OC], f32,
                                    isOutput=False)
    s_d = nc.declare_dram_parameter("scale", [OCH, 128, 1], f32,
                                    isOutput=False)
    o_d = nc.declare_dram_parameter("out", [IMGS, OCH, 128, H, W], f32,
                                    isOutput=True)

    with tile.TileContext(nc) as tc:
        with (
            tc.tile_pool(name="wu", bufs=1) as wup,
            tc.tile_pool(name="wups", bufs=1, space="PSUM") as wupsp,
            tc.tile_pool(name="wr", bufs=1) as wrp,
            tc.tile_pool(name="wc", bufs=1) as wcp,
            tc.tile_pool(name="sp", bufs=1) as sp,
            tc.tile_pool(name="xr", bufs=2) as xrp,
            tc.tile_pool(name="xc", bufs=2) as xcp,
            tc.tile_pool(name="op", bufs=4) as op,
            tc.tile_pool(name="ps", bufs=7, space="PSUM") as psp,
        ):
            # ---- PE warmup: keep the tensor engine busy while inputs load
            wu_raw = wup.tile([128, 64], f32, name="wu_raw")
            wu_sb = wup.tile([128, 64], f32r, name="wu_sb")
            wu_ps = wupsp.tile([64, 64], f32)
            nc.vector.memset(wu_raw[:], 0.0)
            nc.vector.tensor_copy(wu_sb[:], wu_raw[:])
            for _ in range(NWARM):
                nc.tensor.matmul(wu_ps[:], wu_sb[:, 0:64], wu_sb[:, 0:64],
                                 start=True, stop=True)

            PIECES = [(0, 18), (18, 34), (34, 50), (50, HP)]

            def load_image(img, pieces=None, tiles=None):
                """DMA row pieces of one padded image and convert each to
                fp32r as it lands, so early row tiles unblock early."""
                if tiles is None:
                    tiles = (xrp.tile([128, NCH, HP, WP], f32,
                                      name=f"x_raw{img}", tag="x_raw"),
                             xcp.tile([128, NCH, HP, WP], f32r,
                                      name=f"x_sb{img}", tag="x_sb"))
                x_raw, x_sb = tiles
                for a, b in pieces or PIECES:
                    for c in range(NCH):
                        nc.sync.dma_start(x_raw[:, c, a:b],
                                          x_d[img, c, :, a:b])
                    for c in range(NCH):
                        nc.vector.tensor_copy(x_sb[:, c, a:b],
                                              x_raw[:, c, a:b])
                return tiles

            # image 0, rows 0-9 of both chunks: gates the first row tile
            tiles0 = load_image(0, pieces=PIECES[:1])

            # ---- weights: DMA raw, binarize via ACT Sign into fp32r ----
            w_raw = wrp.tile([128, NCH, 3, 3, OC], f32)
            w_sb = wcp.tile([128, NCH, 3, 3, OC], f32r)
            for c in range(NCH):
                for kh in range(3):
                    nc.sync.dma_start(w_raw[:, c, kh],
                                      w_d[c, kh].rearrange("k p o -> p k o"))
                    nc.scalar.activation(w_sb[:, c, kh], w_raw[:, c, kh],
                                         mybir.ActivationFunctionType.Sign)

            # image 0 remaining rows (needed from row tile p=1 onwards)
            x_pending = load_image(0, pieces=PIECES[1:], tiles=tiles0)[1]

            s_sb = sp.tile([128, OCH], f32)
            for oc in range(OCH):
                nc.sync.dma_start(s_sb[:, oc:oc + 1], s_d[oc])

            for img in range(IMGS):
                x_sb = x_pending
                if img + 1 < IMGS:
                    # prefetch next image with whole-chunk DMAs (fewer
                    # descriptor-queue slots; it has a full image of slack)
                    x_pending = load_image(img + 1, pieces=[(0, HP)])[1]

                for oc in range(OCH):
                    for p in range(PT):
                        ps = psp.tile([128, NPIX], f32, name="ps", tag="ps")
                        i = 0
                        for c in range(NCH):
                            for kh in range(3):
                                for kw in range(3):
                                    rhs = x_sb[:, c,
                                               kh + p * RT: kh + p * RT + RT,
                                               kw: kw + W]
                                    lhsT = w_sb[:, c, kh, kw,
                                                oc * 128:(oc + 1) * 128]
                                    nc.tensor.matmul(ps[:], lhsT, rhs,
                                                     start=(i == 0),
                                                     stop=(i == 17))
                                    i += 1
                        o = op.tile([128, RT, W], f32, name="o", tag="o")
                        last = (img == IMGS - 1 and oc == OCH - 1
                                and p == PT - 1)
                        if not last:
                            nc.scalar.activation(
                                o[:], ps[:],
                                mybir.ActivationFunctionType.Copy,
                                scale=s_sb[:, oc:oc + 1])
                            nc.scalar.dma_start(
                                o_d[img, oc, :, p * RT:(p + 1) * RT, :], o[:])
                        else:
                            # final group: epilogue on the idle DVE and store
                            # via the idle sync queue, so the kernel tail does
                            # not queue behind the penultimate group's
                            # ACT-engine epilogue/store
                            nc.vector.tensor_scalar_mul(
                                o[:].rearrange("p a b -> p (a b)"),
                                ps[:], s_sb[:, oc:oc + 1])
                            nc.sync.dma_start(
                                o_d[img, oc, :, p * RT:(p + 1) * RT, :], o[:])

    nc.compile()
    _CACHE["nc"] = nc
    return nc


def kernel(x, weights, real_scaling_factor):
    x = np.ascontiguousarray(x, dtype=np.float32)
    w4 = np.asarray(weights, dtype=np.float32).reshape(OC, IC, 3, 3)
    scale = np.asarray(real_scaling_factor, dtype=np.float32).reshape(OCH, 128, 1)

    # pad to 58x58 and split batch/channel chunks
    xp = np.zeros((N, NCH, 128, HP, WP), dtype=np.float32)
    xp[:, :, :, 1:H + 1, 1:W + 1] = x.reshape(N, NCH, 128, H, W)

    # lhsT layout [ic, oc] per (ic_chunk, kh, kw); raw values (sign on device)
    wt = np.ascontiguousarray(
        w4.transpose(1, 2, 3, 0).reshape(NCH, 128, 3, 3, OC)
          .transpose(0, 2, 3, 1, 4))  # [NCH, 3, 3, 128ic, OC]

    nc = _build()
    in_maps = [
        {"x": xp[i * IMGS:(i + 1) * IMGS], "w": wt, "scale": scale}
        for i in range(NCORES)
    ]
    res = run_bass_kernel_spmd(nc, in_maps, list(range(NCORES)))

    out = np.empty((N, NCH, 128, H, W), dtype=np.float32)
    for i in range(NCORES):
        out[i * IMGS:(i + 1) * IMGS] = res.results[i]["out"]
    return out.reshape(N, OC, H, W)


# revision 37
# speedup vs baseline: 1.0074x; 1.0074x over previous
"""Binarized 3x3 conv (BConv) on 8 TRN2 NeuronCores.

Reference computes: y = conv2d(x, sign(w), stride 1, pad 1) * scale[oc]
with x (32,256,56,56) f32, w (256*256*3*3,1) f32, scale (1,256,1,1) f32.

Strategy: data-parallel over batch (4 images per core, weights + scale
replicated). Per core the conv is lowered to matmuls on the PE array:
for each (ic_chunk, kh, kw), out[oc, pix] += w[ic, oc].T @ x[ic, pix_shifted],
18 accumulating matmuls per PSUM tile of 448 pixels (8 rows x 56).
Inputs are rounded to fp32r on-device (full PE rate at ~2^-13 precision;
binary +-1 weights are exact). Per-out-channel scale is applied by the
ScalarE Copy activation during PSUM evacuation.

Input DMAs issue on the sync engine, output DMAs on the scalar engine so
image prefetch never queues behind result stores. The PE is kept warm
through the initial load phase with throwaway small matmuls.
"""
import numpy as np

import concourse.bacc as bacc
import concourse.mybir as mybir
import concourse.tile as tile
from concourse.bass_utils import run_bass_kernel_spmd

N, IC, OC, H, W = 32, 256, 256, 56, 56
NCORES = 8
IMGS = N // NCORES          # 4 images per core
NCH = IC // 128             # 2 in-channel chunks
OCH = OC // 128             # 2 out-channel chunks
HP, WP = H + 2, W + 2       # padded 58x58
RT = 8                      # output rows per tile
PT = H // RT                # 7 row tiles
NPIX = RT * W               # 448 pixels per matmul
NWARM = 70                  # PE warmup matmuls bridging the load phase

_CACHE = {}


def _build():
    if "nc" in _CACHE:
        return _CACHE["nc"]
    f32 = mybir.dt.float32
    f32r = mybir.dt.float32r
    nc = bacc.Bacc("TRN2", target_bir_lowering=False, debug=False,
                   num_devices=NCORES)

    x_d = nc.declare_dram_parameter("x", [IMGS, NCH, 128, HP, WP], f32,
                                    isOutput=False)
    w_d = nc.declare_dram_parameter("w", [NCH, 3, 3, 128, OC], f32,
                                    isOutput=False)
    s_d = nc.declare_dram_parameter("scale", [OCH, 128, 1], f32,
                                    isOutput=False)
    o_d = nc.declare_dram_parameter("out", [IMGS, OCH, 128, H, W], f32,
                                    isOutput=True)

    with tile.TileContext(nc) as tc:
        with (
            tc.tile_pool(name="wu", bufs=1) as wup,
            tc.tile_pool(name="wups", bufs=1, space="PSUM") as wupsp,
            tc.tile_pool(name="wr", bufs=1) as wrp,
            tc.tile_pool(name="wc", bufs=1) as wcp,
            tc.tile_pool(name="sp", bufs=1) as sp,
            tc.tile_pool(name="xr", bufs=2) as xrp,
            tc.tile_pool(name="xc", bufs=2) as xcp,
            tc.tile_pool(name="op", bufs=4) as op,
            tc.tile_pool(name="ps", bufs=7, space="PSUM") as psp,
        ):
            # ---- PE warmup: keep the tensor engine busy while inputs load
            wu_raw = wup.tile([128, 64], f32, name="wu_raw")
            wu_sb = wup.tile([128, 64], f32r, name="wu_sb")
            wu_ps = wupsp.tile([64, 64], f32)
            nc.vector.memset(wu_raw[:], 0.0)
            nc.vector.tensor_copy(wu_sb[:], wu_raw[:])
            for _ in range(NWARM):
                nc.tensor.matmul(wu_ps[:], wu_sb[:, 0:64], wu_sb[:, 0:64],
                                 start=True, stop=True)

            PIECES = [(0, 18), (18, 34), (34, 50), (50, HP)]

            def load_image(img, pieces=None, tiles=None):
                """DMA row pieces of one padded image and convert each to
                fp32r as it lands, so early row tiles unblock early."""
                if tiles is None:
                    tiles = (xrp.tile([128, NCH, HP, WP], f32,
                                      name=f"x_raw{img}", tag="x_raw"),
                             xcp.tile([128, NCH, HP, WP], f32r,
                                      name=f"x_sb{img}", tag="x_sb"))
                x_raw, x_sb = tiles
                for a, b in pieces or PIECES:
                    for c in range(NCH):
                        nc.sync.dma_start(x_raw[:, c, a:b],
                                          x_d[img, c, :, a:b])
                    for c in range(NCH):
                        nc.vector.tensor_copy(x_sb[:, c, a:b],
                                              x_raw[:, c, a:b])
                return tiles

            w_raw = wrp.tile([128, NCH, 3, 3, OC], f32)
            w_sb = wcp.tile([128, NCH, 3, 3, OC], f32r)

            def load_weights(c):
                # DMA + Sign-binarize one in-channel chunk of the weights
                for kh in range(3):
                    nc.sync.dma_start(w_raw[:, c, kh],
                                      w_d[c, kh].rearrange("k p o -> p k o"))
                    nc.scalar.activation(w_sb[:, c, kh], w_raw[:, c, kh],
                                         mybir.ActivationFunctionType.Sign)

            # image-0 first rows gate the first row tile; then all weights
            # (the first pass consumes them within ~3.4us), then the rest
            tiles0 = load_image(0, pieces=PIECES[:1])
            load_weights(0)
            load_weights(1)
            x_pending = load_image(0, pieces=PIECES[1:], tiles=tiles0)[1]

            s_sb = sp.tile([128, OCH], f32)
            for oc in range(OCH):
                nc.sync.dma_start(s_sb[:, oc:oc + 1], s_d[oc])

            def trim(p, kh):
                # output rows whose kh tap is entirely zero padding are
                # trimmed (saves PE rows; PSUM has_written gives first-
                # write-overwrite for the untouched pixels; fp32r matmuls
                # require contiguous dst, so only whole rows are trimmed)
                ra, rb = 0, RT
                if p == 0 and kh == 0:
                    ra = 1
                if p == PT - 1 and kh == 2:
                    rb = RT - 1
                return ra, rb

            def emit_mm(ps, x_sb, oc, p, c, kh, kw, start, stop):
                ra, rb = trim(p, kh)
                r0 = kh + p * RT + ra
                rhs = x_sb[:, c, r0: r0 + rb - ra, kw: kw + W]
                lhsT = w_sb[:, c, kh, kw, oc * 128:(oc + 1) * 128]
                nc.tensor.matmul(ps[:, ra:rb, :], lhsT, rhs,
                                 start=start, stop=stop)

            def emit_epilogue(ps, img, oc, p, last):
                o = op.tile([128, RT, W], f32, name="o", tag="o")
                if not last:
                    nc.scalar.activation(
                        o[:], ps[:], mybir.ActivationFunctionType.Copy,
                        scale=s_sb[:, oc:oc + 1])
                    nc.scalar.dma_start(
                        o_d[img, oc, :, p * RT:(p + 1) * RT, :], o[:])
                else:
                    # final group: epilogue on the idle DVE and store via
                    # the idle sync queue, so the kernel tail does not queue
                    # behind the penultimate group's ACT-engine work
                    nc.vector.tensor_scalar_mul(
                        o[:].rearrange("p a b -> p (a b)"),
                        ps[:].rearrange("p a b -> p (a b)"),
                        s_sb[:, oc:oc + 1])
                    nc.sync.dma_start(
                        o_d[img, oc, :, p * RT:(p + 1) * RT, :], o[:])

            def emit_group(x_sb, img, oc, p, last=False):
                ps = psp.tile([128, RT, W], f32, name="ps", tag="ps")
                # kh=0 is trimmed for p=0, so emit kh=1 first there: the
                # start=True matmul must cover the full tile (PSUM
                # pending-zero is all-or-nothing per matmul)
                kh_order = (1, 0, 2) if p == 0 else (0, 1, 2)
                i = 0
                for c in range(NCH):
                    for kh in kh_order:
                        for kw in range(3):
                            emit_mm(ps, x_sb, oc, p, c, kh, kw,
                                    i == 0, i == 17)
                            i += 1
                emit_epilogue(ps, img, oc, p, last)

            for img in range(IMGS):
                x_sb = x_pending
                if img + 1 < IMGS:
                    # prefetch next image with whole-chunk DMAs (fewer
                    # descriptor-queue slots; it has a full image of slack)
                    x_pending = load_image(img + 1, pieces=[(0, HP)])[1]

                for oc in range(OCH):
                    for p in range(PT):
                        last = (img == IMGS - 1 and oc == OCH - 1
                                and p == PT - 1)
                        emit_group(x_sb, img, oc, p, last)

    nc.compile()
    _CACHE["nc"] = nc
    return nc


def kernel(x, weights, real_scaling_factor):
    x = np.ascontiguousarray(x, dtype=np.float32)
    w4 = np.asarray(weights, dtype=np.float32).reshape(OC, IC, 3, 3)
    scale = np.asarray(real_scaling_factor, dtype=np.float32).reshape(OCH, 128, 1)

    # pad to 58x58 and split batch/channel chunks
    xp = np.zeros((N, NCH, 128, HP, WP), dtype=np.float32)
    xp[:, :, :, 1:H + 1, 1:W + 1] = x.reshape(N, NCH, 128, H, W)

    # lhsT layout [ic, oc] per (ic_chunk, kh, kw); raw values (sign on device)
    wt = np.ascontiguousarray(
        w4.transpose(1, 2, 3, 0).reshape(NCH, 128, 3, 3, OC)
          .transpose(0, 2, 3, 1, 4))  # [NCH, 3, 3, 128ic, OC]

    nc = _build()
    in_maps = [
        {"x": xp[i * IMGS:(i + 1) * IMGS], "w": wt, "scale": scale}
        for i in range(NCORES)
    ]
    res = run_bass_kernel_spmd(nc, in_maps, list(range(NCORES)))

    out = np.empty((N, NCH, 128, H, W), dtype=np.float32)
    for i in range(NCORES):
        out[i * IMGS:(i + 1) * IMGS] = res.results[i]["out"]
    return out.reshape(N, OC, H, W)


# revision 42
# speedup vs baseline: 1.0246x; 1.0171x over previous
"""Binarized 3x3 conv (BConv) on 8 TRN2 NeuronCores.

Reference computes: y = conv2d(x, sign(w), stride 1, pad 1) * scale[oc]
with x (32,256,56,56) f32, w (256*256*3*3,1) f32, scale (1,256,1,1) f32.

Strategy: data-parallel over batch (4 images per core, weights + scale
replicated). Per core the conv is lowered to matmuls on the PE array:
for each (ic_chunk, kh, kw), out[oc, pix] += w[ic, oc].T @ x[ic, pix_shifted],
18 accumulating matmuls per PSUM tile of 448 pixels (8 rows x 56).
Inputs are rounded to fp32r on-device (full PE rate at ~2^-13 precision;
binary +-1 weights are exact). Per-out-channel scale is applied by the
ScalarE Copy activation during PSUM evacuation.

Input DMAs issue on the sync engine, output DMAs on the scalar engine so
image prefetch never queues behind result stores. The PE is kept warm
through the initial load phase with throwaway small matmuls.
"""
import numpy as np

import concourse.bacc as bacc
import concourse.mybir as mybir
import concourse.tile as tile
from concourse.bass_utils import run_bass_kernel_spmd

N, IC, OC, H, W = 32, 256, 256, 56, 56
NCORES = 8
IMGS = N // NCORES          # 4 images per core
NCH = IC // 128             # 2 in-channel chunks
OCH = OC // 128             # 2 out-channel chunks
HP, WP = H + 2, W + 2       # padded 58x58
RT = 8                      # output rows per tile
PT = H // RT                # 7 row tiles
NPIX = RT * W               # 448 pixels per matmul
NWARM = 47                  # PE warmup matmuls bridging the load phase

_CACHE = {}


def _build():
    if "nc" in _CACHE:
        return _CACHE["nc"]
    f32 = mybir.dt.float32
    f32r = mybir.dt.float32r
    nc = bacc.Bacc("TRN2", target_bir_lowering=False, debug=False,
                   num_devices=NCORES)

    x_d = nc.declare_dram_parameter("x", [IMGS, NCH, 128, HP, WP], f32,
                                    isOutput=False)
    w_d = nc.declare_dram_parameter("w", [NCH, 3, 3, 128, OC], f32,
                                    isOutput=False)
    s_d = nc.declare_dram_parameter("scale", [OCH, 128, 1], f32,
                                    isOutput=False)
    o_d = nc.declare_dram_parameter("out", [IMGS, OCH, 128, H, W], f32,
                                    isOutput=True)

    with tile.TileContext(nc) as tc:
        with (
            tc.tile_pool(name="wu", bufs=1) as wup,
            tc.tile_pool(name="wups", bufs=1, space="PSUM") as wupsp,
            tc.tile_pool(name="wr", bufs=1) as wrp,
            tc.tile_pool(name="wc", bufs=1) as wcp,
            tc.tile_pool(name="sp", bufs=1) as sp,
            tc.tile_pool(name="xr", bufs=2) as xrp,
            tc.tile_pool(name="xc", bufs=2) as xcp,
            tc.tile_pool(name="op", bufs=4) as op,
            tc.tile_pool(name="ps", bufs=7, space="PSUM") as psp,
        ):
            # ---- PE warmup: keep the tensor engine busy while inputs load
            wu_raw = wup.tile([128, 64], f32, name="wu_raw")
            wu_sb = wup.tile([128, 64], f32r, name="wu_sb")
            wu_ps = wupsp.tile([64, 64], f32)
            nc.vector.memset(wu_raw[:], 0.0)
            nc.vector.tensor_copy(wu_sb[:], wu_raw[:])
            for _ in range(NWARM):
                nc.tensor.matmul(wu_ps[:], wu_sb[:, 0:64], wu_sb[:, 0:64],
                                 start=True, stop=True)

            PIECES = [(0, 10), (10, 18), (18, 34), (34, 50), (50, HP)]

            def load_image(img, pieces=None, tiles=None):
                """DMA row pieces of one padded image and convert each to
                fp32r as it lands, so early row tiles unblock early."""
                if tiles is None:
                    tiles = (xrp.tile([128, NCH, HP, WP], f32,
                                      name=f"x_raw{img}", tag="x_raw"),
                             xcp.tile([128, NCH, HP, WP], f32r,
                                      name=f"x_sb{img}", tag="x_sb"))
                x_raw, x_sb = tiles
                for a, b in pieces or PIECES:
                    for c in range(NCH):
                        nc.sync.dma_start(x_raw[:, c, a:b],
                                          x_d[img, c, :, a:b])
                    for c in range(NCH):
                        nc.vector.tensor_copy(x_sb[:, c, a:b],
                                              x_raw[:, c, a:b])
                return tiles

            w_raw = wrp.tile([128, NCH, 3, 3, OC], f32)
            w_sb = wcp.tile([128, NCH, 3, 3, OC], f32r)

            def load_weights(c, oh):
                # DMA + Sign-binarize one (in-chunk, out-half) of the weights
                for kh in range(3):
                    nc.sync.dma_start(
                        w_raw[:, c, kh, :, oh * 128:(oh + 1) * 128],
                        w_d[c, kh, :, :, oh * 128:(oh + 1) * 128]
                           .rearrange("k p o -> p k o"))
                    nc.scalar.activation(
                        w_sb[:, c, kh, :, oh * 128:(oh + 1) * 128],
                        w_raw[:, c, kh, :, oh * 128:(oh + 1) * 128],
                        mybir.ActivationFunctionType.Sign)

            # image-0 rows 0-9 gate row tile 0; then the oc-half-0 weights
            # (all the first pass consumes); rows through 33 cover the row
            # tiles that run while the oc-half-1 weights stream in
            tiles0 = load_image(0, pieces=PIECES[:1])
            load_weights(0, 0)
            load_weights(1, 0)
            load_image(0, pieces=PIECES[1:3], tiles=tiles0)
            load_weights(0, 1)
            load_weights(1, 1)
            x_pending = load_image(0, pieces=PIECES[3:], tiles=tiles0)[1]

            s_sb = sp.tile([128, OCH], f32)
            for oc in range(OCH):
                nc.sync.dma_start(s_sb[:, oc:oc + 1], s_d[oc])

            def trim(p, kh):
                # output rows whose kh tap is entirely zero padding are
                # trimmed (saves PE rows; PSUM has_written gives first-
                # write-overwrite for the untouched pixels; fp32r matmuls
                # require contiguous dst, so only whole rows are trimmed)
                ra, rb = 0, RT
                if p == 0 and kh == 0:
                    ra = 1
                if p == PT - 1 and kh == 2:
                    rb = RT - 1
                return ra, rb

            def emit_mm(ps, x_sb, oc, p, c, kh, kw, start, stop):
                ra, rb = trim(p, kh)
                r0 = kh + p * RT + ra
                rhs = x_sb[:, c, r0: r0 + rb - ra, kw: kw + W]
                lhsT = w_sb[:, c, kh, kw, oc * 128:(oc + 1) * 128]
                nc.tensor.matmul(ps[:, ra:rb, :], lhsT, rhs,
                                 start=start, stop=stop)

            def emit_epilogue(ps, img, oc, p, last):
                o = op.tile([128, RT, W], f32, name="o", tag="o")
                if not last:
                    nc.scalar.activation(
                        o[:], ps[:], mybir.ActivationFunctionType.Copy,
                        scale=s_sb[:, oc:oc + 1])
                    nc.scalar.dma_start(
                        o_d[img, oc, :, p * RT:(p + 1) * RT, :], o[:])
                else:
                    # final group: epilogue on the idle DVE and store via
                    # the idle sync queue, so the kernel tail does not queue
                    # behind the penultimate group's ACT-engine work
                    nc.vector.tensor_scalar_mul(
                        o[:].rearrange("p a b -> p (a b)"),
                        ps[:].rearrange("p a b -> p (a b)"),
                        s_sb[:, oc:oc + 1])
                    nc.sync.dma_start(
                        o_d[img, oc, :, p * RT:(p + 1) * RT, :], o[:])

            def emit_group(x_sb, img, oc, p, last=False):
                ps = psp.tile([128, RT, W], f32, name="ps", tag="ps")
                # kh=0 is trimmed for p=0, so emit kh=1 first there: the
                # start=True matmul must cover the full tile (PSUM
                # pending-zero is all-or-nothing per matmul)
                kh_order = (1, 0, 2) if p == 0 else (0, 1, 2)
                i = 0
                for c in range(NCH):
                    for kh in kh_order:
                        for kw in range(3):
                            emit_mm(ps, x_sb, oc, p, c, kh, kw,
                                    i == 0, i == 17)
                            i += 1
                emit_epilogue(ps, img, oc, p, last)

            for img in range(IMGS):
                x_sb = x_pending
                if img + 1 < IMGS:
                    # prefetch next image with whole-chunk DMAs (fewer
                    # descriptor-queue slots; it has a full image of slack)
                    x_pending = load_image(img + 1, pieces=[(0, HP)])[1]

                for oc in range(OCH):
                    for p in range(PT):
                        last = (img == IMGS - 1 and oc == OCH - 1
                                and p == PT - 1)
                        emit_group(x_sb, img, oc, p, last)

    nc.compile()
    _CACHE["nc"] = nc
    return nc


def kernel(x, weights, real_scaling_factor):
    x = np.ascontiguousarray(x, dtype=np.float32)
    w4 = np.asarray(weights, dtype=np.float32).reshape(OC, IC, 3, 3)
    scale = np.asarray(real_scaling_factor, dtype=np.float32).reshape(OCH, 128, 1)

    # pad to 58x58 and split batch/channel chunks
    xp = np.zeros((N, NCH, 128, HP, WP), dtype=np.float32)
    xp[:, :, :, 1:H + 1, 1:W + 1] = x.reshape(N, NCH, 128, H, W)

    # lhsT layout [ic, oc] per (ic_chunk, kh, kw); raw values (sign on device)
    wt = np.ascontiguousarray(
        w4.transpose(1, 2, 3, 0).reshape(NCH, 128, 3, 3, OC)
          .transpose(0, 2, 3, 1, 4))  # [NCH, 3, 3, 128ic, OC]

    nc = _build()
    in_maps = [
        {"x": xp[i * IMGS:(i + 1) * IMGS], "w": wt, "scale": scale}
        for i in range(NCORES)
    ]
    res = run_bass_kernel_spmd(nc, in_maps, list(range(NCORES)))

    out = np.empty((N, NCH, 128, H, W), dtype=np.float32)
    for i in range(NCORES):
        out[i * IMGS:(i + 1) * IMGS] = res.results[i]["out"]
    return out.reshape(N, OC, H, W)
